# revision 16
# baseline (speedup 1.0000x reference)
"""AdjustHueSaturation Trainium2 kernel (fp16 pipeline, v4 — compile-clean).

Full inputs: imgs (64,3,512,512) f32 in [0,1], xform_params (64,2) f32
(hue delta in [-0.5,0.5], sat scale in [0.2,2]).
Output: (64,3,512,512) f32.

Pure batch data-parallel across 8 NeuronCores (8 images/core). Host
stages imgs as fp16 (halves input DMA) and upcasts the fp16 output back
to f32 (quantization ~2.4e-4, well under the 2e-2 gate).

Per-pixel math (v=maxc, cr=chroma, m=mod(6*hue+6*dh,6), z=m-3):
    cr   = maxc - minc
    icr  = 1/max(cr, 1e-20)                (reciprocal_approx_fast)
    Dsel = (g-b) | (b-r)+2cr | (r-g)-2cr   by argmax channel (b>g>r
           priority; -2cr = +4cr mod 6; ties coincide mod 6)
    e    = Dsel*icr in [-3,3];  z = wrap(e + (6dh-3), [-3,3], period 6)
    c    = min(cr*ds, v);  p = v - c
    t_k  = relu(|z + b_k| - 1),  b_{r,g,b} = 0, +1, -1
    out  = (p + min(t_r,1)*c) | (v - min(t_g,1)*c) | (v - min(t_b,1)*c)

Engine notes (neuronxcc constraints): Pool does only add/sub/mult
tensor_tensor plus tensor_scalar chains (incl. min/max vs scalar);
min/max/compare tensor_tensor, copy_predicated, reciprocal and the
range-wrap custom op live on DVE; Abs/Relu on ACT. copy_predicated
masks must be integer dtype (uint16).
"""

import numpy as np

B, C, H, W = 64, 3, 512, 512
N_CORES = 8
IPC = B // N_CORES          # images per core
P = 128                     # SBUF partitions
FD = (H * W) // P           # 2048 elements per partition per plane
CFD = 1024                  # chunk free-dim
NCH = FD // CFD             # chunks per plane
WBUFS = 6                   # work-tile buffering depth

_nc_cache = {}


def _build_nc():
    from concourse import bass, bacc, mybir
    from concourse.tile import TileContext
    from concourse.dve_ops import ADD_RANGE_WRAP

    f32 = mybir.dt.float32
    f16 = mybir.dt.float16
    u16 = mybir.dt.uint16
    Alu = mybir.AluOpType
    Act = mybir.ActivationFunctionType

    nc = bacc.Bacc()
    for v in (0.0, 1.0, -1.0, 1e-20):
        t_ = nc.alloc_sbuf_tensor(f"constx-{v}", [P, 1], f32)
        nc.gpsimd.memset(t_.ap(), v)
        nc.const_aps.aps[(f32, v)] = t_.ap()
    nc.all_engine_barrier()

    imgs_d = nc.declare_dram_parameter("imgs", [IPC * 3, P, FD], f16, isOutput=False)
    scal_d = nc.declare_dram_parameter("scal", [P, 2 * IPC], f32, isOutput=False)
    out_d = nc.declare_dram_parameter("out", [IPC * 3, P, FD], f16, isOutput=True)

    with TileContext(nc) as tc:
        with tc.tile_pool(name="const", bufs=1) as cpool, \
             tc.tile_pool(name="work", bufs=2) as pool:
            scal_ld = cpool.tile([P, 2 * IPC], f32, name="scal_ld")
            scal_sb = cpool.tile([P, 2 * IPC], f32, name="scal_sb")
            nc.sync.dma_start(out=scal_ld[:, :], in_=scal_d[:, :])
            nc.vector.tensor_copy(scal_sb[:, :], scal_ld[:, :])

            for img in range(IPC):
              ds_ap = scal_sb[:, 2 * img + 0:2 * img + 1]
              hs_ap = scal_sb[:, 2 * img + 1:2 * img + 2]
              for chk in range(NCH):
                lo = chk * CFD
                th = lambda tag, b=WBUFS: pool.tile([P, CFD], f16, tag=tag, name=tag, bufs=b)

                io3 = pool.tile([P, 3, CFD], f16, tag="io3", name="io3", bufs=WBUFS)
                nc.sync.dma_start(
                    out=io3[:, :, :],
                    in_=imgs_d[3 * img:3 * img + 3, :, lo:lo + CFD].rearrange("c p f -> p c f"))
                r, g, b = io3[:, 0, :], io3[:, 1, :], io3[:, 2, :]

                mx1 = th("mx1", 4); maxch = th("maxch"); mn1 = th("mn1", 3)
                d1 = th("d1"); d2 = th("d2"); d3 = th("d3")
                crh = th("crh", 5); crh2 = th("crh2", 3)
                isb = pool.tile([P, CFD], u16, tag="isb", name="isb", bufs=4)
                isg = pool.tile([P, CFD], u16, tag="isg", name="isg", bufs=4)
                cri = pool.tile([P, CFD], f32, tag="cri", name="cri", bufs=4)
                zh = th("zh"); ag = th("ag"); ab = th("ab")
                c_h = th("c_h", 5); p_h = th("p_h")

                # --- diffs (critical path first) ---
                nc.vector.tensor_tensor(d3[:, :], r, g, Alu.subtract)
                nc.gpsimd.tensor_tensor(d2[:, :], b, r, Alu.subtract)
                nc.gpsimd.tensor_tensor(d1[:, :], g, b, Alu.subtract)
                nc.vector.tensor_scalar(isg[:, :], d3[:, :], 0.0, None, Alu.is_lt)

                # --- value / chroma (min/max on DVE, arith on Pool) ---
                nc.vector.tensor_tensor(mx1[:, :], r, g, Alu.max)
                nc.vector.tensor_tensor(maxch[:, :], mx1[:, :], b, Alu.max)
                nc.vector.tensor_tensor(mn1[:, :], r, g, Alu.min)
                nc.vector.tensor_tensor(mn1[:, :], mn1[:, :], b, Alu.min)  # minc
                nc.gpsimd.tensor_tensor(crh[:, :], maxch[:, :], mn1[:, :], Alu.subtract)
                nc.vector.tensor_tensor(isb[:, :], b, mx1[:, :], Alu.is_ge)

                # --- 1/cr (f32; Relu(cr+1e-20) = zero-safe cr) ---
                nc.scalar.activation(cri[:, :], crh[:, :], Act.Relu, bias=1e-20)
                nc.vector.reciprocal_approx_fast(out=cri[:, :], in_=cri[:, :])

                # --- shifted candidates (in-place d2/d3), select into d1 ---
                nc.gpsimd.tensor_scalar(crh2[:, :], crh[:, :], 2.0, None, Alu.mult)
                nc.gpsimd.tensor_tensor(d2[:, :], d2[:, :], crh2[:, :], Alu.add)
                nc.gpsimd.tensor_tensor(d3[:, :], d3[:, :], crh2[:, :], Alu.subtract)
                nc.vector.copy_predicated(d1[:, :], isg[:, :], d2[:, :])
                nc.vector.copy_predicated(d1[:, :], isb[:, :], d3[:, :])

                # --- sat: c = min(cr*ds, v) (Pool ts then DVE min), p = v - c ---
                nc.gpsimd.tensor_scalar(c_h[:, :], crh[:, :], ds_ap, None, Alu.mult)
                nc.vector.tensor_tensor(c_h[:, :], c_h[:, :], maxch[:, :], Alu.min)
                nc.gpsimd.tensor_tensor(p_h[:, :], maxch[:, :], c_h[:, :], Alu.subtract)

                # --- hue: e = Dsel*icr (f16), z = wrap(e + 6dh-3) into [-3,3] ---
                nc.gpsimd.tensor_tensor(zh[:, :], d1[:, :], cri[:, :], Alu.mult)
                nc.vector._custom_dve(
                    ADD_RANGE_WRAP, out=zh[:, :], in0=zh[:, :],
                    s0=hs_ap, s1=3.0, imm2=6.0)

                # --- per-channel a=|z+b_k| (ACT), t=relu(a-1) (ACT) ---
                nc.scalar.activation(ag[:, :], zh[:, :], Act.Abs, bias=1.0)
                nc.scalar.activation(ab[:, :], zh[:, :], Act.Abs, bias=-1.0)
                nc.scalar.activation(zh[:, :], zh[:, :], Act.Abs, bias=0.0)  # ar
                nc.scalar.activation(zh[:, :], zh[:, :], Act.Relu, bias=-1.0)
                nc.scalar.activation(ag[:, :], ag[:, :], Act.Relu, bias=-1.0)
                nc.scalar.activation(ab[:, :], ab[:, :], Act.Relu, bias=-1.0)

                # --- x = min(t,1)*c (ts-min then mult), outs into io3 ---
                nc.vector.tensor_scalar(zh[:, :], zh[:, :], 1.0, None, Alu.min)
                nc.vector.tensor_scalar(ag[:, :], ag[:, :], 1.0, None, Alu.min)
                nc.vector.tensor_scalar(ab[:, :], ab[:, :], 1.0, None, Alu.min)
                nc.vector.tensor_tensor(zh[:, :], zh[:, :], c_h[:, :], Alu.mult)
                nc.vector.tensor_tensor(ag[:, :], ag[:, :], c_h[:, :], Alu.mult)
                nc.gpsimd.tensor_tensor(ab[:, :], ab[:, :], c_h[:, :], Alu.mult)
                nc.gpsimd.tensor_tensor(io3[:, 0, :], p_h[:, :], zh[:, :], Alu.add)
                nc.gpsimd.tensor_tensor(io3[:, 1, :], maxch[:, :], ag[:, :], Alu.subtract)
                nc.gpsimd.tensor_tensor(io3[:, 2, :], maxch[:, :], ab[:, :], Alu.subtract)
                nc.sync.dma_start(
                    out=out_d[3 * img:3 * img + 3, :, lo:lo + CFD].rearrange("c p f -> p c f"),
                    in_=io3[:, :, :])
    nc.finalize()
    return nc


def _make_in_maps(imgs: np.ndarray, xform_params: np.ndarray):
    imgs16 = np.ascontiguousarray(imgs, dtype=np.float16)
    xf = np.asarray(xform_params, dtype=np.float64)
    in_maps = []
    for core in range(N_CORES):
        sl = slice(core * IPC, (core + 1) * IPC)
        shard = imgs16[sl].reshape(IPC * 3, P, FD)
        hue = xf[sl, 0]
        sat = xf[sl, 1]
        scal = np.empty((P, 2 * IPC), dtype=np.float32)
        scal[:, 0::2] = sat[None, :].astype(np.float32)
        scal[:, 1::2] = (6.0 * hue - 3.0)[None, :].astype(np.float32)
        in_maps.append({"imgs": shard, "scal": scal})
    return in_maps


def kernel(imgs: np.ndarray, xform_params: np.ndarray) -> np.ndarray:
    from concourse.bass_utils import run_bass_kernel_spmd

    if "nc" not in _nc_cache:
        _nc_cache["nc"] = _build_nc()
    nc = _nc_cache["nc"]

    in_maps = _make_in_maps(imgs, xform_params)
    res = run_bass_kernel_spmd(nc, in_maps, core_ids=list(range(N_CORES)))
    out = np.empty((B, C, H, W), dtype=np.float32)
    for core in range(N_CORES):
        out[core * IPC:(core + 1) * IPC] = (
            res.results[core]["out"].astype(np.float32).reshape(IPC, C, H, W))
    return out
